# revision 31
# baseline (speedup 1.0000x reference)
"""MoE feed-forward (8 experts, hard argmin routing) on 8 TRN2 NeuronCores.

Strategy
--------
Host (numpy): rms_norm + argmin routing (0.13% of FLOPs), then a dispatch
plan: tokens sorted by expert, padded to 4-token units, packed into a
UNIFORM per-core structure -- every core runs the same static program of
K<=3 expert-segments with identical token counts; only the DATA (which
expert's weights, which tokens) differs per core.  The planner solves a
small exact search: partition per-expert unit-needs into an 8-core x
K-column grid with uniform column sizes, minimizing modeled PE time
(T * 384 cycles + per-matmul pitch floors).  For this routing the 4-token
granularity yields T=1032 cols/core = (476, 308, 248); 8-token units only
reach 1040.  K=4 plans at T=1032 were measured 4-20us SLOWER: the 4th
12.5MB weight reload needs a ~29us prefetch window inside the previous
segment and the stream goes supply-paced.  Weights/activations cast to
bf16 on host (fp32 PSUM accumulation; every fp8 variant exceeds or grazes
the 2e-2 tolerance -- best was down-only fp8 at 1.97e-2).

Device (Bass/Tile, SPMD x8): per segment, stream weights through SBUF in
consumption order.  Up-projection runs ko-outer so each arriving weight
tile is consumed immediately: segment 0 (supply-paced ramp, per-ko [P,1024]
weight tiles + per-ko xn eighths, with ko=0 split into a/g halves and the
ko=0 pa-row issued first so the first matmul needs only ~250KB) holds full
8-PSUM-bank quads; prefetched segments use half-quads (4 banks) so one
half's swiglu (ACT Silu + DVE mul) overlaps the other half's matmuls and
quad boundaries never stall.  Down-projection drains per-round to DRAM in
bf16; the final group drains per-dout-tile to minimize the post-last-matmul
chain.  A ~2us PE warmup (16 matmuls) covers preamble-to-first-data.  All
data DMAs stay on the sync HW DGE queue: the scalar HW queue crawls under
bulk load (232us total when given 5.5MB), gpsimd SW DGE is ~15-50GB/s and
races, and coarser granules than 256KB stall the early rate-limited
(~300GB/s ramp) window.  The BIR patch also drops TileContext's second
end-barrier round + sem range-clear (the runtime teardown re-zeroes every
semaphore anyway), trimming the measured tail.

Host: scatter y back to token order and add the skip connection in fp32.
"""

import json
import math

import ml_dtypes
import numpy as np

N_EXPERTS = 8
DIM = 1024
HID = 2048
N_CORES = 8
P = 128
EPS = 1e-6
UNIT = 2  # token planning granularity (u2 finds K=3 at T=1030; u8 only 1040)
WARMUP_MM = 16  # PE warm-up matmuls before the first data lands (~11us)

BF16 = ml_dtypes.bfloat16


# ----------------------------------------------------------------------------
# BIR fixup: walrus in this container accepts at most ONE sync-wait per
# instruction.  Split instructions with k>1 waits into (k-1) pure-wait
# EventSemaphore instructions on the same engine immediately before.
# ----------------------------------------------------------------------------
def _split_multiwait_json(bir_bytes: bytes) -> bytes:
    m = json.loads(bir_bytes)
    ctr = 0
    for func in m["functions"]:
        for bb in func["blocks"]:
            out = []
            for inst in bb["instructions"]:
                si = inst.get("sync_info")
                waits = (si or {}).get("on_wait") or []
                if len(waits) > 1:
                    for w in waits[:-1]:
                        ctr += 1
                        out.append({
                            "debug": inst.get("debug", 0),
                            "engine": inst["engine"],
                            "ins": [],
                            "outs": [],
                            "name": f"waitfix_{ctr}",
                            "opcode": "EventSemaphore",
                            "sync_info": {"on_update": [], "on_wait": [w]},
                        })
                    si["on_wait"] = [waits[-1]]
                out.append(inst)
            bb["instructions"] = out
    _strip_second_end_barrier(m)
    return json.dumps(m).encode()


def _strip_second_end_barrier(m):
    """TileContext's exit emits [DMA waits, all-engine barrier A, gpsimd
    sem-range-clear, all-engine barrier B].  The runtime's own teardown
    zeroes every semaphore after the program anyway, so the range-clear and
    barrier B only lengthen the measured tail (~0.5us): truncate the end
    block right after barrier A's Pool release (the sem-add-imm update)."""
    for func in m["functions"]:
        for bb in func["blocks"]:
            if not bb.get("name", "").endswith("_end"):
                continue
            insts = bb["instructions"]
            for idx, inst in enumerate(insts):
                if inst["engine"] != "Pool":
                    continue
                for u in ((inst.get("sync_info") or {}).get("on_update")
                          or []):
                    if u.get("update_mode") == "sem-add-imm":
                        bb["instructions"] = insts[: idx + 1]
                        return


def _patch_bass_json(nc):
    orig = nc.to_json_bytes

    def patched():
        return _split_multiwait_json(orig())

    nc.to_json_bytes = patched


# ----------------------------------------------------------------------------
# Host-side routing (replicates the reference numerics in fp32)
# ----------------------------------------------------------------------------
def _route(x, scale, centroids):
    xf = x.reshape(-1, DIM).astype(np.float32)
    ms = np.mean(xf * xf, axis=-1, keepdims=True)
    s = scale.astype(np.float32) / np.sqrt(ms + EPS)
    xn = xf * s
    nx = np.sum(xn * xn, axis=-1)[:, None]
    ny = np.sum(centroids * centroids, axis=-1)[None, :]
    d2 = nx + ny - 2.0 * (xn @ centroids.T)
    ids = np.argmin(d2, axis=-1).astype(np.int32)
    return xn, ids


# ----------------------------------------------------------------------------
# Dispatch planner: uniform per-core segment structure, UNIT-token units.
# Solve: choose K column sizes comp (units, sum=T) and assign each of the
# 8*K cells to an expert (or leave empty) s.t. every expert's cells cover
# its token count.  All cores burn T columns of PE time, so minimize T.
# ----------------------------------------------------------------------------
def _compositions(total, k):
    """Descending compositions of `total` into exactly k positive parts."""
    if k == 1:
        yield (total,)
        return
    for first in range(total - k + 1, 0, -1):
        for rest in _compositions(total - first, k - 1):
            if rest[0] <= first:
                yield (first,) + rest


def _solve_assign(needs, comp, node_budget=20000):
    """Exact backtracking: assign cell multisets (per column) to experts.

    needs: list of (units_needed, expert_id), descending.
    comp: column sizes in units (descending).
    Returns {expert: [count_per_column]} or None.
    """
    K = len(comp)
    avail = [N_CORES] * K
    out = {}
    nodes = [0]

    def cap(av):
        return sum(a * c for a, c in zip(av, comp))

    def expert_combos(v):
        """All (x_0..x_{K-1}) with sum x_j*comp[j] >= v, minimal overshoot
        first, bounded by avail."""
        combos = []

        def rec(j, acc, left):
            if acc >= v:
                combos.append(tuple(left + [0] * (K - j)))
                return
            if j == K:
                return
            # max useful count for this column
            hi = min(avail[j], (v - acc + comp[j] - 1) // comp[j])
            for x in range(hi, -1, -1):
                left.append(x)
                rec(j + 1, acc + x * comp[j], left)
                left.pop()

        rec(0, 0, [])
        combos.sort(key=lambda xs: (sum(x * c for x, c in zip(xs, comp)),
                                    sum(xs)))
        return combos

    def bt(i):
        nodes[0] += 1
        if nodes[0] > node_budget:
            return False
        if i == len(needs):
            return True
        v, e = needs[i]
        rest = sum(n for n, _ in needs[i + 1:])
        for xs in expert_combos(v):
            ok = all(x <= a for x, a in zip(xs, avail))
            if not ok:
                continue
            for j in range(K):
                avail[j] -= xs[j]
            if cap(avail) >= rest and bt(i + 1):
                out[e] = list(xs)
                return True
            for j in range(K):
                avail[j] += xs[j]
        return False

    if bt(0):
        return out
    return None


def _comp_cost(comp_units):
    """Model PE-time (ns) of a composition: per 512-token group, 384 matmuls
    at pitch max(fd cycles @2.4GHz, ~56ns dispatch/LDWEIGHTS floor)."""
    cost = 0.0
    for cu in comp_units:
        L = cu * UNIT
        while L > 0:
            g = min(512, L)
            L -= g
            cost += 384 * max(g * 0.4167, 56.0)
    return cost


def _plan(ids):
    """Returns (comp_tokens, assign, chunks, tok_by_e).

    comp_tokens: tuple of segment sizes in TOKENS (uniform across cores).
    assign: {(core, seg): expert}
    chunks: {(core, seg): n_real_tokens}
    """
    tok_by_e = [np.where(ids == e)[0] for e in range(N_EXPERTS)]
    needs_u = [(len(t) + UNIT - 1) // UNIT for t in tok_by_e]
    total_u = sum(needs_u)
    lb = max((total_u + N_CORES - 1) // N_CORES,
             (max(needs_u) + N_CORES - 1) // N_CORES if needs_u else 1)
    needs = sorted(((n, e) for e, n in enumerate(needs_u) if n > 0),
                   reverse=True)

    # K<=3 strongly preferred: K=4 at T=1032 was measured 191.4us vs K=3's
    # ~187us -- the 4th 12.5MB weight reload makes every segment's prefetch
    # window tight (~29us needed) and the stream runs ~7us supply-paced,
    # dwarfing the 1.3us PE saving.  Min segment size: 136 tokens (the
    # 56ns LDWEIGHTS pitch floor) for K<=3; 176 (prefetch window) for K=4.
    min_part = {2: 136 // UNIT, 3: 136 // UNIT, 4: 176 // UNIT}
    for kset in ((2, 3), (4,)):
        for T in range(lb, lb + 2 * (P // UNIT) + 2):
            cands = []
            for K in kset:
                if K > T:
                    continue
                for comp in _compositions(T, K):
                    if min(comp) >= min_part[K]:
                        cands.append(comp)
            # cheapest modeled PE time first; first feasible wins
            cands.sort(key=_comp_cost)
            for comp in cands[:3000]:
                K = len(comp)
                sol = _solve_assign(needs, comp, node_budget=20000)
                if sol is None:
                    continue
                # order segments: largest first (good weight-stream ramp),
                # smallest last (small drain tail)
                order = sorted(range(K), key=lambda j: -comp[j])
                comp2 = tuple(comp[j] * UNIT for j in order)
                # materialize cells -> (core, seg) slots
                assign = {}
                chunks = {}
                next_core = [0] * K
                for v, e in needs:
                    remaining = len(tok_by_e[e])
                    # fill this expert's cells largest-column-first
                    cells = []
                    for j in range(K):
                        for _ in range(sol[e][j]):
                            cells.append(j)
                    cells.sort(key=lambda j: -comp[j])
                    for j in cells:
                        c = next_core[j]
                        next_core[j] += 1
                        newj = order.index(j)
                        take = min(comp[j] * UNIT, remaining)
                        assign[(c, newj)] = e
                        chunks[(c, newj)] = take
                        remaining -= take
                    assert remaining == 0
                return comp2, assign, chunks, tok_by_e
    raise RuntimeError("dispatch packing failed")


# ----------------------------------------------------------------------------
# Device program
# ----------------------------------------------------------------------------
def _build_program(comp):
    import concourse.bass as bass
    import concourse.mybir as mybir
    import concourse.tile as tile

    f32 = mybir.dt.float32
    bf16 = mybir.dt.bfloat16
    Silu = mybir.ActivationFunctionType.Silu

    K = len(comp)
    T = sum(comp)  # token slots per core

    nc = bass.Bass("TRN2", debug=False)
    # xn, segment-contiguous: segment s occupies cols [8*col_s, 8*(col_s+
    # comp_s)) laid out ko-major ([8, comp_s] flattened) -> one big-row DMA
    # per segment instead of 8 strided ones.
    xnt_in = nc.dram_tensor("xnt", [P, 8 * T], bf16,
                            kind="ExternalInput").ap()
    # up weights: per (segment, j-quad q of 4, ko-quad kq of 2): [128, 4, 1024]
    # where the last dim = cols [a(4q)..a(4q+3) | g(4q)..g(4q+3)] per ko.
    up_in = nc.dram_tensor("up", [K, 4, 2, P, 4, 1024], bf16,
                           kind="ExternalInput").ap()
    # segment-0 quad-0 fine block, ko-major contiguous (16KB rows) so the
    # ramp moves at full queue rate with only 4 descriptor issues.
    upq0_in = nc.dram_tensor("upq0", [P, 8192], bf16,
                             kind="ExternalInput").ap()
    # down weights: per (segment, kh-quad kq of 4): [128, 4, 1024]
    # (1024 = all 8 dout tiles) per kh.
    down_in = nc.dram_tensor("down", [K, 4, P, 4, 1024], bf16,
                             kind="ExternalInput").ap()
    yt_out = nc.dram_tensor("yt", [P, 8, T], bf16, kind="ExternalOutput").ap()

    with tile.TileContext(nc) as tc:
        with (
            tc.tile_pool(name="upw", bufs=8) as up_pool,
            tc.tile_pool(name="upf", bufs=8) as upf_pool,
            tc.tile_pool(name="dnw", bufs=4) as dn_pool,
            tc.tile_pool(name="xn", bufs=2) as xn_pool,
            tc.tile_pool(name="xnf", bufs=4) as xnf_pool,
            tc.tile_pool(name="xn8", bufs=2) as xn8_pool,
            tc.tile_pool(name="act", bufs=2) as act_pool,
            tc.tile_pool(name="yc", bufs=1) as yc_pool,
            tc.tile_pool(name="ps", bufs=8, space="PSUM") as ps,
        ):
            # PE warm-up: dependency-free matmuls on a zeroed scratch tile
            # cover the HAM clock ramp while the first DMAs land.  Sized to
            # finish just as the first weight/xn tiles land (the warmup's two
            # PSUM banks are reused by the first full quad, so overshooting
            # delays the first data matmul).
            with tc.tile_pool(name="warm", bufs=1) as warm_pool:
                # 32B ring-prime: the sync HW DGE's first transfer pays a
                # ~0.9us ring-open latency; paying it on this no-op pull
                # lets the first real weight tile stream at issue+0.
                wprime = warm_pool.tile([1, 16], bf16, tag="wprime")
                nc.sync.dma_start(wprime[:], xnt_in[0:1, 0:16])
                wsrc = warm_pool.tile([P, 256], bf16, tag="warm")
                nc.gpsimd.memset(wsrc[:], 0.0)
                wps = [ps.tile([P, P], f32, tag="ps", name=f"wps{i}")
                       for i in range(2)]
                for i in range(WARMUP_MM):
                    nc.tensor.matmul(wps[i % 2][:], wsrc[:, 0:P],
                                     wsrc[:, P : 2 * P],
                                     start=True, stop=True)

            col = 0
            for s in range(K):
                # the whole segment's activations FIRST (small; the first
                # matmul needs them, so they must not queue behind weights)
                seg_tok = comp[s]
                xbase = 8 * col
                if s == 0:
                    # first segment, issue order tuned for the ~600ns-per-
                    # DMA sync-queue issue rate and ~0.9us ring-open latency:
                    # tiny first dependencies (a-half of ko=0 + xn eighth),
                    # then progressively COARSER transfers -- small transfers
                    # throttle the early stream to ~280GB/s (one 256KB
                    # transfer per ~625ns issue slot) while 0.5-1MB
                    # transfers run at the ~430GB/s queue peak.
                    # (the scalar HW DGE queue was tried for the first loads
                    # and measured ~3x slower than sync's; everything stays
                    # on the sync queue)
                    wf0_halves = [None, None]  # [a-half, g-half] of ko=0
                    wfa = upf_pool.tile([P, 512], bf16, tag="upf0a",
                                        bufs=1, name="upf_0a")
                    nc.sync.dma_start(wfa[:], upq0_in[:, 0:512])
                    wf0_halves[0] = wfa
                    xe0 = xn8_pool.tile([P, seg_tok], bf16, tag="xn8",
                                        name="xn8_0")
                    nc.sync.dma_start(xe0[:],
                                      xnt_in[:, xbase : xbase + seg_tok])
                    wfg = upf_pool.tile([P, 512], bf16, tag="upf0g",
                                        bufs=1, name="upf_0g")
                    nc.sync.dma_start(wfg[:], upq0_in[:, 512:1024])
                    wf0_halves[1] = wfg
                    xe1 = xn8_pool.tile([P, seg_tok], bf16, tag="xn8",
                                        name="xn8_1")
                    nc.sync.dma_start(
                        xe1[:],
                        xnt_in[:, xbase + seg_tok : xbase + 2 * seg_tok])
                    wf1 = upf_pool.tile([P, 1024], bf16, tag="upf1",
                                        bufs=1, name="upf_1")
                    nc.sync.dma_start(wf1[:], upq0_in[:, 1024:2048])
                    # per-ko 256KB fine tiles from here: the early stream is
                    # rate-limited (~300GB/s ramp) regardless of transfer
                    # size, and coarser granules (one 1MB ko4-7 tile) were
                    # measured to STALL ko4 and push the q1 quads later.
                    # (Offloading the odd tiles to the scalar HW DGE queue
                    # was measured at 232us total -- that queue crawls under
                    # bulk load; everything stays on sync.)
                    up_fine0 = [None] * 8
                    xn_parts = [None] * 4
                    for kp in range(1, 4):
                        for ko in (2 * kp, 2 * kp + 1):
                            wf = upf_pool.tile([P, 1024], bf16, tag="upff",
                                               bufs=6, name=f"upf_{ko}")
                            nc.sync.dma_start(
                                wf[:], upq0_in[:, 1024 * ko :
                                               1024 * (ko + 1)])
                            up_fine0[ko] = wf
                            if ko == 2 * kp:
                                xk = xnf_pool.tile([P, 2 * seg_tok], bf16,
                                                   tag="xnf",
                                                   name=f"xn0_{kp}")
                                nc.sync.dma_start(
                                    xk[:],
                                    xnt_in[:, xbase + 2 * kp * seg_tok :
                                           xbase + 2 * (kp + 1) * seg_tok])
                                xn_parts[kp] = xk

                    def get_xn(ko, xslc, st=seg_tok):
                        if ko == 0:
                            return xe0[:, xslc]
                        if ko == 1:
                            return xe1[:, xslc]
                        part = xn_parts[ko // 2]
                        off = (ko % 2) * st
                        return part[:, off + xslc.start : off + xslc.stop]
                else:
                    xn_seg = xn_pool.tile([P, 8 * seg_tok], bf16, tag="xn")
                    nc.sync.dma_start(
                        xn_seg[:],
                        xnt_in[:, xbase : xbase + 8 * seg_tok])

                    def get_xn(ko, xslc, xn_seg=xn_seg, st=seg_tok):
                        off = ko * st
                        return xn_seg[:, off + xslc.start : off + xslc.stop]
                # resident weights for this segment, in consumption order
                upt = {}
                for q in range(4):
                    if s == 0 and q == 0:
                        continue  # all 8 ko loaded fine above
                    for kq in range(2):
                        w = up_pool.tile([P, 4, 1024], bf16, tag="upw",
                                         name=f"up_{s}_{q}_{kq}")
                        nc.sync.dma_start(w[:], up_in[s, q, kq])
                        upt[(q, kq)] = w

                def get_up_col(s_, q, ko, c0, upt=upt):
                    """[P, 128] weight column block (c0 in [0, 1024))."""
                    if s_ == 0 and q == 0:
                        if ko == 0:
                            half = wf0_halves[c0 >= 512]
                            return half[:, c0 % 512 : c0 % 512 + P]
                        if ko == 1:
                            return wf1[:, c0 : c0 + P]
                        return up_fine0[ko][:, c0 : c0 + P]
                    return upt[(q, ko // 4)][:, ko % 4, c0 : c0 + P]

                dnt = {}
                for kq in range(4):
                    w = dn_pool.tile([P, 4, 1024], bf16, tag="dnw",
                                     name=f"dn_{s}_{kq}")
                    nc.sync.dma_start(w[:], down_in[s, kq])
                    dnt[kq] = w

                rem = seg_tok
                segcol = 0
                while rem > 0:
                    gn = min(512, rem)
                    rem -= gn
                    xslc = slice(segcol, segcol + gn)
                    segcol += gn
                    act_t = act_pool.tile([P, 16, gn], bf16, tag="act")
                    # ---- up projection: ko-outer / sub-inner so each fresh
                    # weight tile is consumed as soon as it lands.  Segment 0
                    # (supply-paced ramp) runs full quads (8 banks, 8 mm per
                    # tile); later segments run HALF-quads (4 banks) so one
                    # half's Silu/mul overlaps the other half's matmuls and
                    # quad boundaries never stall on PSUM-bank frees ----
                    for q in range(4):
                        # seg0's q0/q1 run full quads (8 mms per arriving
                        # tile -- their weight stream is just-in-time); by q2
                        # the stream is back-pressured/resident, so half-quad
                        # double-buffering is safe everywhere else
                        sub_sets = ([(0, 1, 2, 3)] if (s == 0 and q < 2)
                                    else [(0, 1), (2, 3)])
                        for subs in sub_sets:
                            pa = {i: ps.tile([P, gn], f32, tag="ps",
                                             name=f"pa{i}") for i in subs}
                            pg = {i: ps.tile([P, gn], f32, tag="ps",
                                             name=f"pg{i}") for i in subs}
                            for ko in range(8):
                                xr = get_xn(ko, xslc)
                                first, last = ko == 0, ko == 7
                                if s == 0 and q == 0 and ko == 0:
                                    # pa row first: those 4 matmuls depend
                                    # only on the a-half of the split ko=0
                                    # tile, so compute starts ~0.3us before
                                    # the g-half lands
                                    for sub in subs:
                                        nc.tensor.matmul(
                                            pa[sub][:],
                                            get_up_col(s, q, ko, sub * P),
                                            xr, start=first, stop=last)
                                    for sub in subs:
                                        nc.tensor.matmul(
                                            pg[sub][:],
                                            get_up_col(s, q, ko,
                                                       512 + sub * P),
                                            xr, start=first, stop=last)
                                    continue
                                for sub in subs:
                                    ca = sub * P
                                    cg = 512 + sub * P
                                    nc.tensor.matmul(
                                        pa[sub][:],
                                        get_up_col(s, q, ko, ca),
                                        xr, start=first, stop=last)
                                    nc.tensor.matmul(
                                        pg[sub][:],
                                        get_up_col(s, q, ko, cg),
                                        xr, start=first, stop=last)
                            for sub in subs:
                                j = 4 * q + sub
                                nc.scalar.activation(act_t[:, j, :],
                                                     pg[sub][:], Silu)
                                nc.vector.tensor_mul(act_t[:, j, :],
                                                     pa[sub][:],
                                                     act_t[:, j, :])
                    # ---- down projection: 4 rounds x (16 kh x 2 matmuls) ----
                    # last group gets its own (small) yc tile so its copies
                    # never wait on the previous group's output DMA
                    is_last = (s == K - 1 and rem == 0)
                    yc = yc_pool.tile([P, 8, gn], bf16,
                                      tag="ycl" if is_last else "yc",
                                      name="yc")
                    # last group drains per-dout (1-bank rounds) so the
                    # post-final-matmul chain is one copy + a half-size DMA
                    ndout = 1 if is_last else 2
                    nround = 8 // ndout - (1 if is_last else 0)
                    for rr in range(nround):
                        pd = [ps.tile([P, gn], f32, tag="ps", name=f"pd{q}")
                              for q in range(ndout)]
                        for kh in range(16):
                            w = dnt[kh // 4][:, kh % 4, :]
                            first, last = kh == 0, kh == 15
                            for q in range(ndout):
                                c = (ndout * rr + q) * P
                                nc.tensor.matmul(pd[q][:], w[:, c : c + P],
                                                 act_t[:, kh, :], start=first, stop=last)
                        for q in range(ndout):
                            nc.vector.tensor_copy(yc[:, ndout * rr + q, :],
                                                  pd[q][:])
                        # per-round output DMA overlaps later rounds' matmuls
                        nc.sync.dma_start(
                            yt_out[:, ndout * rr : ndout * rr + ndout,
                                   col : col + gn],
                            yc[:, ndout * rr : ndout * rr + ndout, :])
                    if is_last:
                        # the very last dout runs in two COLUMN halves so the
                        # post-final-matmul chain is a half-size CAST + a
                        # 31KB DMA (the first half's cast/DMA overlap the
                        # second half's 16 matmuls)
                        h = gn // 2
                        for ci, (c0, c1) in enumerate(((0, h), (h, gn))):
                            pdh = ps.tile([P, c1 - c0], f32, tag="ps",
                                          name=f"pdh{ci}")
                            for kh in range(16):
                                w = dnt[kh // 4][:, kh % 4, :]
                                nc.tensor.matmul(
                                    pdh[:], w[:, 7 * P : 8 * P],
                                    act_t[:, kh, c0:c1],
                                    start=kh == 0, stop=kh == 15)
                            nc.vector.tensor_copy(yc[:, 7, c0:c1], pdh[:])
                            nc.sync.dma_start(
                                yt_out[:, 7:8, col + c0 : col + c1],
                                yc[:, 7:8, c0:c1])
                    col += gn

    _patch_bass_json(nc)
    return nc


# ----------------------------------------------------------------------------
# Host-side weight packing into the streaming layouts
# ----------------------------------------------------------------------------
def _pack_up(up_e_bf):
    """[DIM, 2H] bf16 -> [4 q, 2 kq, 128, 4 koi, 1024]."""
    U = up_e_bf.reshape(8, P, 2 * HID)
    A = U[:, :, :HID].reshape(8, P, 16, P)
    G = U[:, :, HID:].reshape(8, P, 16, P)
    out = np.empty((4, 8, P, 1024), dtype=BF16)
    for q in range(4):
        for i in range(4):
            out[q, :, :, i * P : (i + 1) * P] = A[:, :, 4 * q + i]
            out[q, :, :, 512 + i * P : 512 + (i + 1) * P] = G[:, :, 4 * q + i]
    # [4, 8ko, P, 1024] -> [4, 2, 4, P, 1024] -> [4, 2, P, 4, 1024]
    return np.ascontiguousarray(
        out.reshape(4, 2, 4, P, 1024).transpose(0, 1, 3, 2, 4)
    )


def _pack_down(down_e_bf):
    """[HID, DIM] bf16 -> [4 kq, 128, 4 khi, 1024]."""
    D = down_e_bf.reshape(4, 4, P, DIM)
    return np.ascontiguousarray(D.transpose(0, 2, 1, 3))


# ----------------------------------------------------------------------------
# Entry point
# ----------------------------------------------------------------------------
def _run(inputs, trace=False, tmpdir=None):
    from concourse.bass_utils import run_bass_kernel_spmd

    x = np.asarray(inputs["x"])
    scale = np.asarray(inputs["scale"])
    centroids = np.asarray(inputs["centroids"])
    up_w = np.asarray(inputs["up_w"])
    down_w = np.asarray(inputs["down_w"])

    B, S, D = x.shape
    ntok = B * S
    xf32 = x.reshape(ntok, D).astype(np.float32)

    xn, ids = _route(x, scale, centroids)
    comp, assign, chunks, tok_by_e = _plan(ids)
    K = len(comp)
    T = sum(comp)

    # pre-pack each expert's weights once (experts can appear on many cores)
    up_packed_e = {}
    down_packed_e = {}
    for e in range(N_EXPERTS):
        if any(v == e for v in assign.values()):
            up_packed_e[e] = _pack_up(up_w[e].astype(BF16))
            down_packed_e[e] = _pack_down(down_w[e].astype(BF16))

    xnT = np.ascontiguousarray(xn.T)  # [DIM, ntok] f32
    cursor = [0] * N_EXPERTS
    core_cols_tok = [np.zeros(T, dtype=np.int64) for _ in range(N_CORES)]
    core_cols_valid = [np.zeros(T, dtype=bool) for _ in range(N_CORES)]
    in_maps = []
    for c in range(N_CORES):
        up_pack = np.zeros((K, 4, 2, P, 4, 1024), dtype=BF16)
        upq0_pack = np.zeros((P, 8192), dtype=BF16)
        down_pack = np.zeros((K, 4, P, 4, 1024), dtype=BF16)
        col = 0
        for j, sz in enumerate(comp):
            e = assign.get((c, j))
            if e is not None:
                up_pack[j] = up_packed_e[e]
                down_pack[j] = down_packed_e[e]
                if j == 0:
                    # [2 kq, P, 4 koi, 1024] -> [P, 8 ko, 1024] contiguous
                    upq0_pack[:] = np.ascontiguousarray(
                        up_packed_e[e][0].transpose(1, 0, 2, 3)
                    ).reshape(P, 8192)
                toks = tok_by_e[e]
                take = min(chunks[(c, j)], len(toks) - cursor[e])
                take = max(take, 0)
                if take:
                    sel = toks[cursor[e] : cursor[e] + take]
                    cursor[e] += take
                    core_cols_tok[c][col : col + take] = sel
                    core_cols_valid[c][col : col + take] = True
            col += sz
        xnt_cols = xnT[:, core_cols_tok[c]].astype(BF16)  # [DIM, T]
        # segment-contiguous layout: [P, sum_s 8*comp_s], segment block
        # laid out ko-major ([8, comp_s] flattened per partition row)
        xnt_pack = np.zeros((P, 8 * T), dtype=BF16)
        col = 0
        for sz in comp:
            blk = xnt_cols[:, col : col + sz].reshape(8, P, sz)
            xnt_pack[:, 8 * col : 8 * (col + sz)] = (
                blk.transpose(1, 0, 2).reshape(P, 8 * sz)
            )
            col += sz
        in_maps.append({"xnt": xnt_pack, "up": up_pack,
                        "upq0": upq0_pack, "down": down_pack})

    for e in range(N_EXPERTS):
        assert cursor[e] == len(tok_by_e[e]), "dispatch did not cover all tokens"

    nc = _build_program(comp)
    kwargs = {}
    if trace:
        kwargs = dict(trace=True, tmpdir=tmpdir)
    res = run_bass_kernel_spmd(nc, in_maps, core_ids=list(range(N_CORES)), **kwargs)

    # ---- scatter + skip ----
    out = xf32.copy()
    for c in range(N_CORES):
        # yt_out layout is [P, 8 dout-tiles, T]; dout index = do*128 + p
        yt = np.ascontiguousarray(
            res.results[c]["yt"].reshape(P, 8, T).transpose(1, 0, 2)
        ).reshape(8 * P, T).astype(np.float32)  # [DIM, T]
        valid = core_cols_valid[c]
        toks = core_cols_tok[c][valid]
        out[toks] = xf32[toks] + yt[:, valid].T
    return out.reshape(B, S, D).astype(x.dtype), res


def kernel(**inputs) -> np.ndarray:
    out, _ = _run(inputs)
    return out



# revision 32
# speedup vs baseline: 1.0158x; 1.0158x over previous
"""MoE feed-forward (8 experts, hard argmin routing) on 8 TRN2 NeuronCores.

Strategy
--------
Host (numpy): rms_norm + argmin routing (0.13% of FLOPs), then a dispatch
plan: tokens sorted by expert, padded to 4-token units, packed into a
UNIFORM per-core structure -- every core runs the same static program of
K<=3 expert-segments with identical token counts; only the DATA (which
expert's weights, which tokens) differs per core.  The planner solves a
small exact search: partition per-expert unit-needs into an 8-core x
K-column grid with uniform column sizes, minimizing modeled PE time
(T * 384 cycles + per-matmul pitch floors).  For this routing the 4-token
granularity yields T=1032 cols/core = (476, 308, 248); 8-token units only
reach 1040.  K=4 plans at T=1032 were measured 4-20us SLOWER: the 4th
12.5MB weight reload needs a ~29us prefetch window inside the previous
segment and the stream goes supply-paced.  Weights/activations cast to
bf16 on host (fp32 PSUM accumulation; every fp8 variant exceeds or grazes
the 2e-2 tolerance -- best was down-only fp8 at 1.97e-2).

Device (Bass/Tile, SPMD x8): per segment, stream weights through SBUF in
consumption order.  Up-projection runs ko-outer so each arriving weight
tile is consumed immediately: segment 0 (supply-paced ramp, per-ko [P,1024]
weight tiles + per-ko xn eighths, with ko=0 split into a/g halves and the
ko=0 pa-row issued first so the first matmul needs only ~250KB) holds full
8-PSUM-bank quads; prefetched segments use half-quads (4 banks) so one
half's swiglu (ACT Silu + DVE mul) overlaps the other half's matmuls and
quad boundaries never stall.  Down-projection drains per-round to DRAM in
bf16; the final group drains per-dout-tile to minimize the post-last-matmul
chain.  A ~2us PE warmup (16 matmuls) covers preamble-to-first-data.  All
data DMAs stay on the sync HW DGE queue: the scalar HW queue crawls under
bulk load (232us total when given 5.5MB), gpsimd SW DGE is ~15-50GB/s and
races, and coarser granules than 256KB stall the early rate-limited
(~300GB/s ramp) window.  The BIR patch also drops TileContext's second
end-barrier round + sem range-clear (the runtime teardown re-zeroes every
semaphore anyway), trimming the measured tail.

Host: scatter y back to token order and add the skip connection in fp32.
"""

import json
import math

import ml_dtypes
import numpy as np

N_EXPERTS = 8
DIM = 1024
HID = 2048
N_CORES = 8
P = 128
EPS = 1e-6
UNIT = 4  # token planning granularity (u4 finds K=3 at T=1032; u8 only 1040)
WARMUP_MM = 16  # PE warm-up matmuls before the first data lands (~11us)

BF16 = ml_dtypes.bfloat16


# ----------------------------------------------------------------------------
# BIR fixup: walrus in this container accepts at most ONE sync-wait per
# instruction.  Split instructions with k>1 waits into (k-1) pure-wait
# EventSemaphore instructions on the same engine immediately before.
# ----------------------------------------------------------------------------
def _split_multiwait_json(bir_bytes: bytes) -> bytes:
    m = json.loads(bir_bytes)
    ctr = 0
    for func in m["functions"]:
        for bb in func["blocks"]:
            out = []
            for inst in bb["instructions"]:
                si = inst.get("sync_info")
                waits = (si or {}).get("on_wait") or []
                if len(waits) > 1:
                    for w in waits[:-1]:
                        ctr += 1
                        out.append({
                            "debug": inst.get("debug", 0),
                            "engine": inst["engine"],
                            "ins": [],
                            "outs": [],
                            "name": f"waitfix_{ctr}",
                            "opcode": "EventSemaphore",
                            "sync_info": {"on_update": [], "on_wait": [w]},
                        })
                    si["on_wait"] = [waits[-1]]
                out.append(inst)
            bb["instructions"] = out
    _strip_second_end_barrier(m)
    return json.dumps(m).encode()


def _strip_second_end_barrier(m):
    """TileContext's exit emits [DMA waits, all-engine barrier A, gpsimd
    sem-range-clear, all-engine barrier B].  The runtime's own teardown
    zeroes every semaphore after the program anyway, so the range-clear and
    barrier B only lengthen the measured tail (~0.5us): truncate the end
    block right after barrier A's Pool release (the sem-add-imm update)."""
    for func in m["functions"]:
        for bb in func["blocks"]:
            if not bb.get("name", "").endswith("_end"):
                continue
            insts = bb["instructions"]
            for idx, inst in enumerate(insts):
                if inst["engine"] != "Pool":
                    continue
                for u in ((inst.get("sync_info") or {}).get("on_update")
                          or []):
                    if u.get("update_mode") == "sem-add-imm":
                        bb["instructions"] = insts[: idx + 1]
                        return


def _patch_bass_json(nc):
    orig = nc.to_json_bytes

    def patched():
        return _split_multiwait_json(orig())

    nc.to_json_bytes = patched


# ----------------------------------------------------------------------------
# Host-side routing (replicates the reference numerics in fp32)
# ----------------------------------------------------------------------------
def _route(x, scale, centroids):
    xf = x.reshape(-1, DIM).astype(np.float32)
    ms = np.mean(xf * xf, axis=-1, keepdims=True)
    s = scale.astype(np.float32) / np.sqrt(ms + EPS)
    xn = xf * s
    nx = np.sum(xn * xn, axis=-1)[:, None]
    ny = np.sum(centroids * centroids, axis=-1)[None, :]
    d2 = nx + ny - 2.0 * (xn @ centroids.T)
    ids = np.argmin(d2, axis=-1).astype(np.int32)
    return xn, ids


# ----------------------------------------------------------------------------
# Dispatch planner: uniform per-core segment structure, UNIT-token units.
# Solve: choose K column sizes comp (units, sum=T) and assign each of the
# 8*K cells to an expert (or leave empty) s.t. every expert's cells cover
# its token count.  All cores burn T columns of PE time, so minimize T.
# ----------------------------------------------------------------------------
def _compositions(total, k):
    """Descending compositions of `total` into exactly k positive parts."""
    if k == 1:
        yield (total,)
        return
    for first in range(total - k + 1, 0, -1):
        for rest in _compositions(total - first, k - 1):
            if rest[0] <= first:
                yield (first,) + rest


def _solve_assign(needs, comp, node_budget=20000):
    """Exact backtracking: assign cell multisets (per column) to experts.

    needs: list of (units_needed, expert_id), descending.
    comp: column sizes in units (descending).
    Returns {expert: [count_per_column]} or None.
    """
    K = len(comp)
    avail = [N_CORES] * K
    out = {}
    nodes = [0]

    def cap(av):
        return sum(a * c for a, c in zip(av, comp))

    def expert_combos(v):
        """All (x_0..x_{K-1}) with sum x_j*comp[j] >= v, minimal overshoot
        first, bounded by avail."""
        combos = []

        def rec(j, acc, left):
            if acc >= v:
                combos.append(tuple(left + [0] * (K - j)))
                return
            if j == K:
                return
            # max useful count for this column
            hi = min(avail[j], (v - acc + comp[j] - 1) // comp[j])
            for x in range(hi, -1, -1):
                left.append(x)
                rec(j + 1, acc + x * comp[j], left)
                left.pop()

        rec(0, 0, [])
        combos.sort(key=lambda xs: (sum(x * c for x, c in zip(xs, comp)),
                                    sum(xs)))
        return combos

    def bt(i):
        nodes[0] += 1
        if nodes[0] > node_budget:
            return False
        if i == len(needs):
            return True
        v, e = needs[i]
        rest = sum(n for n, _ in needs[i + 1:])
        for xs in expert_combos(v):
            ok = all(x <= a for x, a in zip(xs, avail))
            if not ok:
                continue
            for j in range(K):
                avail[j] -= xs[j]
            if cap(avail) >= rest and bt(i + 1):
                out[e] = list(xs)
                return True
            for j in range(K):
                avail[j] += xs[j]
        return False

    if bt(0):
        return out
    return None


def _comp_cost(comp_units):
    """Model PE-time (ns) of a composition: per 512-token group, 384 matmuls
    at pitch max(fd cycles @2.4GHz, ~56ns dispatch/LDWEIGHTS floor)."""
    cost = 0.0
    for cu in comp_units:
        L = cu * UNIT
        while L > 0:
            g = min(512, L)
            L -= g
            cost += 384 * max(g * 0.4167, 56.0)
    return cost


def _plan(ids):
    """Returns (comp_tokens, assign, chunks, tok_by_e).

    comp_tokens: tuple of segment sizes in TOKENS (uniform across cores).
    assign: {(core, seg): expert}
    chunks: {(core, seg): n_real_tokens}
    """
    tok_by_e = [np.where(ids == e)[0] for e in range(N_EXPERTS)]
    needs_u = [(len(t) + UNIT - 1) // UNIT for t in tok_by_e]
    total_u = sum(needs_u)
    lb = max((total_u + N_CORES - 1) // N_CORES,
             (max(needs_u) + N_CORES - 1) // N_CORES if needs_u else 1)
    needs = sorted(((n, e) for e, n in enumerate(needs_u) if n > 0),
                   reverse=True)

    # K<=3 strongly preferred: K=4 at T=1032 was measured 191.4us vs K=3's
    # ~187us -- the 4th 12.5MB weight reload makes every segment's prefetch
    # window tight (~29us needed) and the stream runs ~7us supply-paced,
    # dwarfing the 1.3us PE saving.  Min segment size: 136 tokens (the
    # 56ns LDWEIGHTS pitch floor) for K<=3; 176 (prefetch window) for K=4.
    min_part = {2: 136 // UNIT, 3: 136 // UNIT, 4: 176 // UNIT}
    for kset in ((2, 3), (4,)):
        for T in range(lb, lb + 2 * (P // UNIT) + 2):
            cands = []
            for K in kset:
                if K > T:
                    continue
                for comp in _compositions(T, K):
                    if min(comp) >= min_part[K]:
                        cands.append(comp)
            # cheapest modeled PE time first; first feasible wins
            cands.sort(key=_comp_cost)
            for comp in cands[:800]:
                K = len(comp)
                sol = _solve_assign(needs, comp, node_budget=20000)
                if sol is None:
                    continue
                # order segments: largest first (good weight-stream ramp),
                # smallest last (small drain tail)
                order = sorted(range(K), key=lambda j: -comp[j])
                comp2 = tuple(comp[j] * UNIT for j in order)
                # materialize cells -> (core, seg) slots
                assign = {}
                chunks = {}
                next_core = [0] * K
                for v, e in needs:
                    remaining = len(tok_by_e[e])
                    # fill this expert's cells largest-column-first
                    cells = []
                    for j in range(K):
                        for _ in range(sol[e][j]):
                            cells.append(j)
                    cells.sort(key=lambda j: -comp[j])
                    for j in cells:
                        c = next_core[j]
                        next_core[j] += 1
                        newj = order.index(j)
                        take = min(comp[j] * UNIT, remaining)
                        assign[(c, newj)] = e
                        chunks[(c, newj)] = take
                        remaining -= take
                    assert remaining == 0
                return comp2, assign, chunks, tok_by_e
    raise RuntimeError("dispatch packing failed")


# ----------------------------------------------------------------------------
# Device program
# ----------------------------------------------------------------------------
def _build_program(comp):
    import concourse.bass as bass
    import concourse.mybir as mybir
    import concourse.tile as tile

    f32 = mybir.dt.float32
    bf16 = mybir.dt.bfloat16
    Silu = mybir.ActivationFunctionType.Silu

    K = len(comp)
    T = sum(comp)  # token slots per core

    nc = bass.Bass("TRN2", debug=False)
    # xn, segment-contiguous: segment s occupies cols [8*col_s, 8*(col_s+
    # comp_s)) laid out ko-major ([8, comp_s] flattened) -> one big-row DMA
    # per segment instead of 8 strided ones.
    xnt_in = nc.dram_tensor("xnt", [P, 8 * T], bf16,
                            kind="ExternalInput").ap()
    # up weights: per (segment, j-quad q of 4, ko-quad kq of 2): [128, 4, 1024]
    # where the last dim = cols [a(4q)..a(4q+3) | g(4q)..g(4q+3)] per ko.
    up_in = nc.dram_tensor("up", [K, 4, 2, P, 4, 1024], bf16,
                           kind="ExternalInput").ap()
    # segment-0 quad-0 fine block, ko-major contiguous (16KB rows) so the
    # ramp moves at full queue rate with only 4 descriptor issues.
    upq0_in = nc.dram_tensor("upq0", [P, 8192], bf16,
                             kind="ExternalInput").ap()
    # down weights: per (segment, kh-quad kq of 4): [128, 4, 1024]
    # (1024 = all 8 dout tiles) per kh.
    down_in = nc.dram_tensor("down", [K, 4, P, 4, 1024], bf16,
                             kind="ExternalInput").ap()
    yt_out = nc.dram_tensor("yt", [P, 8, T], bf16, kind="ExternalOutput").ap()

    with tile.TileContext(nc) as tc:
        with (
            tc.tile_pool(name="upw", bufs=8) as up_pool,
            tc.tile_pool(name="upf", bufs=8) as upf_pool,
            tc.tile_pool(name="dnw", bufs=4) as dn_pool,
            tc.tile_pool(name="xn", bufs=2) as xn_pool,
            tc.tile_pool(name="xnf", bufs=4) as xnf_pool,
            tc.tile_pool(name="xn8", bufs=2) as xn8_pool,
            tc.tile_pool(name="act", bufs=2) as act_pool,
            tc.tile_pool(name="yc", bufs=1) as yc_pool,
            tc.tile_pool(name="ps", bufs=8, space="PSUM") as ps,
        ):
            # PE warm-up: dependency-free matmuls on a zeroed scratch tile
            # cover the HAM clock ramp while the first DMAs land.  Sized to
            # finish just as the first weight/xn tiles land (the warmup's two
            # PSUM banks are reused by the first full quad, so overshooting
            # delays the first data matmul).
            with tc.tile_pool(name="warm", bufs=1) as warm_pool:
                # 32B ring-prime: the sync HW DGE's first transfer pays a
                # ~0.9us ring-open latency; paying it on this no-op pull
                # lets the first real weight tile stream at issue+0.
                wprime = warm_pool.tile([1, 16], bf16, tag="wprime")
                nc.sync.dma_start(wprime[:], xnt_in[0:1, 0:16])
                wsrc = warm_pool.tile([P, 256], bf16, tag="warm")
                nc.gpsimd.memset(wsrc[:], 0.0)
                wps = [ps.tile([P, P], f32, tag="ps", name=f"wps{i}")
                       for i in range(2)]
                for i in range(WARMUP_MM):
                    nc.tensor.matmul(wps[i % 2][:], wsrc[:, 0:P],
                                     wsrc[:, P : 2 * P],
                                     start=True, stop=True)

            col = 0
            for s in range(K):
                # the whole segment's activations FIRST (small; the first
                # matmul needs them, so they must not queue behind weights)
                seg_tok = comp[s]
                xbase = 8 * col
                if s == 0:
                    # first segment, issue order tuned for the ~600ns-per-
                    # DMA sync-queue issue rate and ~0.9us ring-open latency:
                    # tiny first dependencies (a-half of ko=0 + xn eighth),
                    # then progressively COARSER transfers -- small transfers
                    # throttle the early stream to ~280GB/s (one 256KB
                    # transfer per ~625ns issue slot) while 0.5-1MB
                    # transfers run at the ~430GB/s queue peak.
                    # (the scalar HW DGE queue was tried for the first loads
                    # and measured ~3x slower than sync's; everything stays
                    # on the sync queue)
                    wf0_halves = [None, None]  # [a-half, g-half] of ko=0
                    wfa = upf_pool.tile([P, 512], bf16, tag="upf0a",
                                        bufs=1, name="upf_0a")
                    nc.sync.dma_start(wfa[:], upq0_in[:, 0:512])
                    wf0_halves[0] = wfa
                    xe0 = xn8_pool.tile([P, seg_tok], bf16, tag="xn8",
                                        name="xn8_0")
                    nc.sync.dma_start(xe0[:],
                                      xnt_in[:, xbase : xbase + seg_tok])
                    wfg = upf_pool.tile([P, 512], bf16, tag="upf0g",
                                        bufs=1, name="upf_0g")
                    nc.sync.dma_start(wfg[:], upq0_in[:, 512:1024])
                    wf0_halves[1] = wfg
                    xe1 = xn8_pool.tile([P, seg_tok], bf16, tag="xn8",
                                        name="xn8_1")
                    nc.sync.dma_start(
                        xe1[:],
                        xnt_in[:, xbase + seg_tok : xbase + 2 * seg_tok])
                    wf1 = upf_pool.tile([P, 1024], bf16, tag="upf1",
                                        bufs=1, name="upf_1")
                    nc.sync.dma_start(wf1[:], upq0_in[:, 1024:2048])
                    # per-ko 256KB fine tiles from here: the early stream is
                    # rate-limited (~300GB/s ramp) regardless of transfer
                    # size, and coarser granules (one 1MB ko4-7 tile) were
                    # measured to STALL ko4 and push the q1 quads later.
                    # (Offloading the odd tiles to the scalar HW DGE queue
                    # was measured at 232us total -- that queue crawls under
                    # bulk load; everything stays on sync.)
                    up_fine0 = [None] * 8
                    xn_parts = [None] * 4
                    for kp in range(1, 4):
                        for ko in (2 * kp, 2 * kp + 1):
                            wf = upf_pool.tile([P, 1024], bf16, tag="upff",
                                               bufs=6, name=f"upf_{ko}")
                            nc.sync.dma_start(
                                wf[:], upq0_in[:, 1024 * ko :
                                               1024 * (ko + 1)])
                            up_fine0[ko] = wf
                            if ko == 2 * kp:
                                xk = xnf_pool.tile([P, 2 * seg_tok], bf16,
                                                   tag="xnf",
                                                   name=f"xn0_{kp}")
                                nc.sync.dma_start(
                                    xk[:],
                                    xnt_in[:, xbase + 2 * kp * seg_tok :
                                           xbase + 2 * (kp + 1) * seg_tok])
                                xn_parts[kp] = xk

                    def get_xn(ko, xslc, st=seg_tok):
                        if ko == 0:
                            return xe0[:, xslc]
                        if ko == 1:
                            return xe1[:, xslc]
                        part = xn_parts[ko // 2]
                        off = (ko % 2) * st
                        return part[:, off + xslc.start : off + xslc.stop]
                else:
                    xn_seg = xn_pool.tile([P, 8 * seg_tok], bf16, tag="xn")
                    nc.sync.dma_start(
                        xn_seg[:],
                        xnt_in[:, xbase : xbase + 8 * seg_tok])

                    def get_xn(ko, xslc, xn_seg=xn_seg, st=seg_tok):
                        off = ko * st
                        return xn_seg[:, off + xslc.start : off + xslc.stop]
                # resident weights for this segment, in consumption order
                upt = {}
                for q in range(4):
                    if s == 0 and q == 0:
                        continue  # all 8 ko loaded fine above
                    for kq in range(2):
                        w = up_pool.tile([P, 4, 1024], bf16, tag="upw",
                                         name=f"up_{s}_{q}_{kq}")
                        nc.sync.dma_start(w[:], up_in[s, q, kq])
                        upt[(q, kq)] = w

                def get_up_col(s_, q, ko, c0, upt=upt):
                    """[P, 128] weight column block (c0 in [0, 1024))."""
                    if s_ == 0 and q == 0:
                        if ko == 0:
                            half = wf0_halves[c0 >= 512]
                            return half[:, c0 % 512 : c0 % 512 + P]
                        if ko == 1:
                            return wf1[:, c0 : c0 + P]
                        return up_fine0[ko][:, c0 : c0 + P]
                    return upt[(q, ko // 4)][:, ko % 4, c0 : c0 + P]

                dnt = {}
                for kq in range(4):
                    w = dn_pool.tile([P, 4, 1024], bf16, tag="dnw",
                                     name=f"dn_{s}_{kq}")
                    nc.sync.dma_start(w[:], down_in[s, kq])
                    dnt[kq] = w

                rem = seg_tok
                segcol = 0
                while rem > 0:
                    gn = min(512, rem)
                    rem -= gn
                    xslc = slice(segcol, segcol + gn)
                    segcol += gn
                    act_t = act_pool.tile([P, 16, gn], bf16, tag="act")
                    # ---- up projection: ko-outer / sub-inner so each fresh
                    # weight tile is consumed as soon as it lands.  Segment 0
                    # (supply-paced ramp) runs full quads (8 banks, 8 mm per
                    # tile); later segments run HALF-quads (4 banks) so one
                    # half's Silu/mul overlaps the other half's matmuls and
                    # quad boundaries never stall on PSUM-bank frees ----
                    for q in range(4):
                        # seg0's q0/q1 run full quads (8 mms per arriving
                        # tile -- their weight stream is just-in-time); by q2
                        # the stream is back-pressured/resident, so half-quad
                        # double-buffering is safe everywhere else
                        sub_sets = ([(0, 1, 2, 3)] if (s == 0 and q < 2)
                                    else [(0, 1), (2, 3)])
                        for subs in sub_sets:
                            pa = {i: ps.tile([P, gn], f32, tag="ps",
                                             name=f"pa{i}") for i in subs}
                            pg = {i: ps.tile([P, gn], f32, tag="ps",
                                             name=f"pg{i}") for i in subs}
                            for ko in range(8):
                                xr = get_xn(ko, xslc)
                                first, last = ko == 0, ko == 7
                                if s == 0 and q == 0 and ko == 0:
                                    # pa row first: those 4 matmuls depend
                                    # only on the a-half of the split ko=0
                                    # tile, so compute starts ~0.3us before
                                    # the g-half lands
                                    for sub in subs:
                                        nc.tensor.matmul(
                                            pa[sub][:],
                                            get_up_col(s, q, ko, sub * P),
                                            xr, start=first, stop=last)
                                    for sub in subs:
                                        nc.tensor.matmul(
                                            pg[sub][:],
                                            get_up_col(s, q, ko,
                                                       512 + sub * P),
                                            xr, start=first, stop=last)
                                    continue
                                for sub in subs:
                                    ca = sub * P
                                    cg = 512 + sub * P
                                    nc.tensor.matmul(
                                        pa[sub][:],
                                        get_up_col(s, q, ko, ca),
                                        xr, start=first, stop=last)
                                    nc.tensor.matmul(
                                        pg[sub][:],
                                        get_up_col(s, q, ko, cg),
                                        xr, start=first, stop=last)
                            for sub in subs:
                                j = 4 * q + sub
                                nc.scalar.activation(act_t[:, j, :],
                                                     pg[sub][:], Silu)
                                nc.vector.tensor_mul(act_t[:, j, :],
                                                     pa[sub][:],
                                                     act_t[:, j, :])
                    # ---- down projection: 4 rounds x (16 kh x 2 matmuls) ----
                    # last group gets its own (small) yc tile so its copies
                    # never wait on the previous group's output DMA
                    is_last = (s == K - 1 and rem == 0)
                    yc = yc_pool.tile([P, 8, gn], bf16,
                                      tag="ycl" if is_last else "yc",
                                      name="yc")
                    # last group drains per-dout (1-bank rounds) so the
                    # post-final-matmul chain is one copy + a half-size DMA
                    ndout = 1 if is_last else 2
                    nround = 8 // ndout
                    for rr in range(nround):
                        pd = [ps.tile([P, gn], f32, tag="ps", name=f"pd{q}")
                              for q in range(ndout)]
                        for kh in range(16):
                            w = dnt[kh // 4][:, kh % 4, :]
                            first, last = kh == 0, kh == 15
                            for q in range(ndout):
                                c = (ndout * rr + q) * P
                                nc.tensor.matmul(pd[q][:], w[:, c : c + P],
                                                 act_t[:, kh, :], start=first, stop=last)
                        for q in range(ndout):
                            nc.vector.tensor_copy(yc[:, ndout * rr + q, :],
                                                  pd[q][:])
                        # per-round output DMA overlaps later rounds' matmuls
                        nc.sync.dma_start(
                            yt_out[:, ndout * rr : ndout * rr + ndout,
                                   col : col + gn],
                            yc[:, ndout * rr : ndout * rr + ndout, :])
                    col += gn

    _patch_bass_json(nc)
    return nc


# ----------------------------------------------------------------------------
# Host-side weight packing into the streaming layouts
# ----------------------------------------------------------------------------
def _pack_up(up_e_bf):
    """[DIM, 2H] bf16 -> [4 q, 2 kq, 128, 4 koi, 1024]."""
    U = up_e_bf.reshape(8, P, 2 * HID)
    A = U[:, :, :HID].reshape(8, P, 16, P)
    G = U[:, :, HID:].reshape(8, P, 16, P)
    out = np.empty((4, 8, P, 1024), dtype=BF16)
    for q in range(4):
        for i in range(4):
            out[q, :, :, i * P : (i + 1) * P] = A[:, :, 4 * q + i]
            out[q, :, :, 512 + i * P : 512 + (i + 1) * P] = G[:, :, 4 * q + i]
    # [4, 8ko, P, 1024] -> [4, 2, 4, P, 1024] -> [4, 2, P, 4, 1024]
    return np.ascontiguousarray(
        out.reshape(4, 2, 4, P, 1024).transpose(0, 1, 3, 2, 4)
    )


def _pack_down(down_e_bf):
    """[HID, DIM] bf16 -> [4 kq, 128, 4 khi, 1024]."""
    D = down_e_bf.reshape(4, 4, P, DIM)
    return np.ascontiguousarray(D.transpose(0, 2, 1, 3))


# ----------------------------------------------------------------------------
# Entry point
# ----------------------------------------------------------------------------
def _run(inputs, trace=False, tmpdir=None):
    from concourse.bass_utils import run_bass_kernel_spmd

    x = np.asarray(inputs["x"])
    scale = np.asarray(inputs["scale"])
    centroids = np.asarray(inputs["centroids"])
    up_w = np.asarray(inputs["up_w"])
    down_w = np.asarray(inputs["down_w"])

    B, S, D = x.shape
    ntok = B * S
    xf32 = x.reshape(ntok, D).astype(np.float32)

    xn, ids = _route(x, scale, centroids)
    comp, assign, chunks, tok_by_e = _plan(ids)
    K = len(comp)
    T = sum(comp)

    # pre-pack each expert's weights once (experts can appear on many cores)
    up_packed_e = {}
    down_packed_e = {}
    for e in range(N_EXPERTS):
        if any(v == e for v in assign.values()):
            up_packed_e[e] = _pack_up(up_w[e].astype(BF16))
            down_packed_e[e] = _pack_down(down_w[e].astype(BF16))

    xnT = np.ascontiguousarray(xn.T)  # [DIM, ntok] f32
    cursor = [0] * N_EXPERTS
    core_cols_tok = [np.zeros(T, dtype=np.int64) for _ in range(N_CORES)]
    core_cols_valid = [np.zeros(T, dtype=bool) for _ in range(N_CORES)]
    in_maps = []
    for c in range(N_CORES):
        up_pack = np.zeros((K, 4, 2, P, 4, 1024), dtype=BF16)
        upq0_pack = np.zeros((P, 8192), dtype=BF16)
        down_pack = np.zeros((K, 4, P, 4, 1024), dtype=BF16)
        col = 0
        for j, sz in enumerate(comp):
            e = assign.get((c, j))
            if e is not None:
                up_pack[j] = up_packed_e[e]
                down_pack[j] = down_packed_e[e]
                if j == 0:
                    # [2 kq, P, 4 koi, 1024] -> [P, 8 ko, 1024] contiguous
                    upq0_pack[:] = np.ascontiguousarray(
                        up_packed_e[e][0].transpose(1, 0, 2, 3)
                    ).reshape(P, 8192)
                toks = tok_by_e[e]
                take = min(chunks[(c, j)], len(toks) - cursor[e])
                take = max(take, 0)
                if take:
                    sel = toks[cursor[e] : cursor[e] + take]
                    cursor[e] += take
                    core_cols_tok[c][col : col + take] = sel
                    core_cols_valid[c][col : col + take] = True
            col += sz
        xnt_cols = xnT[:, core_cols_tok[c]].astype(BF16)  # [DIM, T]
        # segment-contiguous layout: [P, sum_s 8*comp_s], segment block
        # laid out ko-major ([8, comp_s] flattened per partition row)
        xnt_pack = np.zeros((P, 8 * T), dtype=BF16)
        col = 0
        for sz in comp:
            blk = xnt_cols[:, col : col + sz].reshape(8, P, sz)
            xnt_pack[:, 8 * col : 8 * (col + sz)] = (
                blk.transpose(1, 0, 2).reshape(P, 8 * sz)
            )
            col += sz
        in_maps.append({"xnt": xnt_pack, "up": up_pack,
                        "upq0": upq0_pack, "down": down_pack})

    for e in range(N_EXPERTS):
        assert cursor[e] == len(tok_by_e[e]), "dispatch did not cover all tokens"

    nc = _build_program(comp)
    kwargs = {}
    if trace:
        kwargs = dict(trace=True, tmpdir=tmpdir)
    res = run_bass_kernel_spmd(nc, in_maps, core_ids=list(range(N_CORES)), **kwargs)

    # ---- scatter + skip ----
    out = xf32.copy()
    for c in range(N_CORES):
        # yt_out layout is [P, 8 dout-tiles, T]; dout index = do*128 + p
        yt = np.ascontiguousarray(
            res.results[c]["yt"].reshape(P, 8, T).transpose(1, 0, 2)
        ).reshape(8 * P, T).astype(np.float32)  # [DIM, T]
        valid = core_cols_valid[c]
        toks = core_cols_tok[c][valid]
        out[toks] = xf32[toks] + yt[:, valid].T
    return out.reshape(B, S, D).astype(x.dtype), res


def kernel(**inputs) -> np.ndarray:
    out, _ = _run(inputs)
    return out

